# revision 28
# baseline (speedup 1.0000x reference)
"""Pairwise-cosine loss kernel for Trainium2 (8 NeuronCores, Bass/Tile).

Reference computation:
  - box geometry features from bboxes -> small MLP -> shape_feat [1024,6,64]
  - combined = concat(feat_embedding, shape_feat) in box-major row order -> [6144, 128]
  - cos = gram / max(outer(norms), eps); sims = cos[jnp.triu_indices(N, 1)]
  - loss = CosineEmbeddingLoss semantics (identically 0 for target==sims)

Device computes the RAW gram blocks in fp32r (RNE-11-bit mantissa matmul);
the host normalizes with the gram diagonal exactly like the reference.

Sharding: core c owns row-blocks r = 8k + c (k = 0..5, 128 rows each).
Slot k computes cols [1024k : 6144) of its row-block -- identical static
write addresses on every core (SPMD); which rows a slot holds is carried
by the per-core input data (own rows appended to the feature inputs).
"""
import sys

for _p in ("/opt/trn_rl_repo",):
    if _p not in sys.path:
        sys.path.insert(0, _p)

import numpy as np
from contextlib import ExitStack

import concourse.mybir as mybir
import concourse.tile as tile
from concourse import bacc
from concourse.bass_utils import run_bass_kernel_spmd

F32 = mybir.dt.float32
F32R = mybir.dt.float32r

BS, NB, DF = 1024, 6, 64
N = BS * NB          # 6144 rows total
NCORES = 8
SLOTS = 6            # row-blocks per core
RB = 128             # row-block size
NT = N // RB         # 48 full tiles
MYN = SLOTS * RB     # 768 own rows
TOT = N + MYN        # 6912 columns processed on device
CH = 512             # matmul free-dim chunk
NTT = NT + SLOTS     # 54 tiles of 128 rows

# wpack layout (cols)
_WCOL = {"w1": 0, "w2": 32, "wh1": 96, "wh2": 160, "wres": 224, "wint": 288, "wout": 352}
_BCOL = {"b1": 416, "b2": 417, "bh1": 418, "bh2": 419, "bres": 420, "bint": 421, "bout": 422}
WPACK_W = 423
DEBUG = False

# column-chunk groups: each group runs the full MLP then its gram+writes, so
# output DMA overlaps the next group's MLP.  Own-rows chunks (gram lhsT) go
# first; high columns next (they carry the most gram output rows).
CHUNK_GROUPS = [
    [6144, 6656, 5632, 5120, 4608, 4096],
    [3584, 3072, 2560, 2048],
    [1536, 1024, 512, 0],
]
# box-feature tile groups (4 tiles each), own-rows tiles first
_TR_GROUPS = [(48, 4), (52, 2)] + [(4 * g, 4) for g in range(12)]

RELU = mybir.ActivationFunctionType.Relu
IDENT = mybir.ActivationFunctionType.Identity
ADD = mybir.AluOpType.add
MAX = mybir.AluOpType.max


def _build():
    nc = bacc.Bacc("TRN2", target_bir_lowering=False, debug=False, num_devices=NCORES)

    featxT = nc.declare_dram_parameter("featxT", [DF, TOT], F32R, isOutput=False)
    bboxx = nc.declare_dram_parameter("bboxx", [128, NTT, 4], F32, isOutput=False)
    wpk = nc.declare_dram_parameter("wpack", [128, WPACK_W], F32, isOutput=False)
    idin = nc.declare_dram_parameter("ident", [128, 128], F32, isOutput=False)
    cos_part = nc.declare_dram_parameter("cos_part", [MYN, N], F32, isOutput=True)
    if DEBUG:
        dbg_comb = nc.declare_dram_parameter("dbg_comb", [128, TOT], F32, isOutput=True)
        dbg_bhT = nc.declare_dram_parameter("dbg_bhT", [6, TOT], F32, isOutput=True)

    with tile.TileContext(nc) as tc, ExitStack() as ctx:
        singles = ctx.enter_context(tc.tile_pool(name="singles", bufs=1))
        work = ctx.enter_context(tc.tile_pool(name="work", bufs=2))
        stage = ctx.enter_context(tc.tile_pool(name="stage", bufs=6))
        ps_tr = ctx.enter_context(tc.tile_pool(name="ps_tr", bufs=2, space="PSUM"))
        ps_mlp = ctx.enter_context(tc.tile_pool(name="ps_mlp", bufs=3, space="PSUM"))
        ps_g = ctx.enter_context(tc.tile_pool(name="ps_g", bufs=3, space="PSUM"))

        # ---- persistent SBUF tensors ----
        combT = singles.tile([128, TOT], F32R)    # rows 0:64 featT, 64:128 shapeT
        box_hwT = singles.tile([6, TOT], F32R)    # geometry features, transposed
        hA = singles.tile([64, TOT], F32R)        # layer ping buffer (h1/h4/h6)
        hB = singles.tile([64, TOT], F32R)        # layer pong buffer (h3/hid)
        h2f = singles.tile([64, TOT], F32R)       # h2 kept whole (residual input)
        ident = singles.tile([128, 128], F32)
        wp = singles.tile([128, WPACK_W], F32)
        wpr = singles.tile([128, WPACK_W], F32R)  # fp32r-rounded weights for PE
        bbox_all = singles.tile([128, NTT, 4], F32)
        # geometry, row-major; 6 features padded to 32 so 4-tile transpose
        # groups land each tile at a 32-aligned PSUM partition offset
        bh = singles.tile([128, NTT, 32], F32)

        nc.sync.dma_start(bbox_all, bboxx[:, :, :])
        nc.sync.dma_start(ident, idin[:, :])
        nc.sync.dma_start(wp, wpk[:, :])
        nc.vector.tensor_copy(wpr, wp)
        # feat rows arrive host-transposed and host-pre-rounded to fp32r bits
        nc.sync.dma_start(combT[0:64, :], featxT[:, :])

        # ---- box geometry: [w, h, w*h, w/h, w+h, sqrt(w^2+h^2)] ----
        w_ = bh[:, :, 0]
        h_ = bh[:, :, 1]
        nc.vector.tensor_sub(w_, bbox_all[:, :, 2], bbox_all[:, :, 0])
        nc.vector.tensor_sub(h_, bbox_all[:, :, 3], bbox_all[:, :, 1])
        nc.vector.tensor_mul(bh[:, :, 2], w_, h_)
        rh = work.tile([128, NTT], F32)
        nc.vector.reciprocal(rh, h_)
        nc.vector.tensor_mul(bh[:, :, 3], w_, rh)
        nc.vector.tensor_add(bh[:, :, 4], w_, h_)
        ww = work.tile([128, NTT], F32)
        hh = work.tile([128, NTT], F32)
        nc.vector.tensor_mul(ww, w_, w_)
        nc.vector.tensor_mul(hh, h_, h_)
        nc.vector.tensor_add(ww, ww, hh)
        nc.scalar.sqrt(bh[:, :, 5], ww)

        # ---- transpose geometry into box_hwT [6, TOT]; own-rows tiles first ----
        ci = 0
        for tlo, gsz in _TR_GROUPS:
            pst = ps_tr.tile([128, 128], F32, tag="tr")
            nc.tensor.transpose(pst[: gsz * 32, :], bh[:, tlo : tlo + gsz, :], ident)
            for dt in range(gsz):
                t = tlo + dt
                dst = box_hwT[:, t * RB : (t + 1) * RB]
                src = pst[dt * 32 : dt * 32 + 6, :]
                if ci % 2 == 0:
                    nc.vector.tensor_copy(dst, src)
                else:
                    nc.scalar.copy(dst, src)
                ci += 1

        # ---- MLP, breadth-first over layers; epilogues alternate ACT/DVE ----
        def wap(name):
            kd = {"w1": 6, "w2": 32}.get(name, 64)
            return wpr[0:kd, _WCOL[name] : _WCOL[name] + (32 if name == "w1" else 64)]

        def bap(name, odim):
            return wp[0:odim, _BCOL[name] : _BCOL[name] + 1]

        def layer(chunks, wname, bname, src_ap_fn, dst, odim, act=True):
            for i, c0 in enumerate(chunks):
                cw = min(CH, TOT - c0)
                sl = slice(c0, c0 + cw)
                ps = ps_mlp.tile([64, CH], F32, tag="mlp")
                nc.tensor.matmul(ps[:odim, :cw], wap(wname), src_ap_fn(sl, cw))
                if i % 2 == 0:
                    nc.scalar.activation(dst[0:odim, sl], ps[:odim, :cw],
                                         RELU if act else IDENT, bias=bap(bname, odim))
                else:
                    if act:
                        nc.vector.tensor_scalar(dst[0:odim, sl], ps[:odim, :cw],
                                                bap(bname, odim), 0.0, ADD, MAX)
                    else:
                        nc.vector.tensor_scalar(dst[0:odim, sl], ps[:odim, :cw],
                                                bap(bname, odim), None, ADD)

        gi = 0
        for chunks in CHUNK_GROUPS:
            layer(chunks, "w1", "b1", lambda sl, cw: box_hwT[:, sl], hA, 32)
            layer(chunks, "w2", "b2", lambda sl, cw: hA[0:32, sl], h2f, 64)
            layer(chunks, "wh1", "bh1", lambda sl, cw: h2f[:, sl], hB, 64)
            layer(chunks, "wh2", "bh2", lambda sl, cw: hB[:, sl], hA, 64)
            # hid = h4 + (h2 @ wres); bres is folded into bint on the host
            for c0 in chunks:
                cw = min(CH, TOT - c0)
                sl = slice(c0, c0 + cw)
                ps = ps_mlp.tile([64, CH], F32, tag="mlp")
                nc.tensor.matmul(ps[:, :cw], wap("wres"), h2f[:, sl])
                nc.vector.tensor_tensor(hB[:, sl], ps[:, :cw],
                                        hA[0:64, sl].bitcast(F32), ADD)
            layer(chunks, "wint", "bint", lambda sl, cw: hB[:, sl], hA, 64)
            layer(chunks, "wout", "bout", lambda sl, cw: hA[0:64, sl],
                  combT[64:128, :], 64, act=False)

            # gram + output for this group's columns (own-rows chunks excluded)
            for c0 in chunks:
                if c0 >= N:
                    continue
                for k in range(min(c0 // 1024, SLOTS - 1) + 1):
                    lhsT = combT[:, N + k * RB : N + (k + 1) * RB]
                    psg = ps_g.tile([128, CH], F32, tag="g")
                    nc.tensor.matmul(psg, lhsT, combT[:, c0 : c0 + CH])
                    st = stage.tile([128, CH], F32, tag="st")
                    if gi % 3 == 0:
                        nc.vector.tensor_copy(st, psg)
                    else:
                        nc.scalar.copy(st, psg)
                    gi += 1
                    nc.sync.dma_start(cos_part[k * RB : (k + 1) * RB, c0 : c0 + CH], st)

        if DEBUG:
            nc.sync.dma_start(dbg_comb[:, :], combT.bitcast(F32))
            nc.sync.dma_start(dbg_bhT[:, :], box_hwT.bitcast(F32))

    nc.finalize()
    return nc


_NC_CACHE = {}


def _get_nc():
    if "nc" not in _NC_CACHE:
        _NC_CACHE["nc"] = _build()
    return _NC_CACHE["nc"]


def _triu_indices_like_reference():
    """Reproduce the exact index sequence the reference's jnp.triu_indices
    emits (it differs from numpy's: fp rounding inside jax's nonzero-based
    implementation shifts ~3M entries, yields j=-1 and lower-triangle pairs).
    Computed on the CPU backend, as the single-device jax reference does."""
    if "triu" not in _NC_CACHE:
        import jax

        with jax.default_device(jax.devices("cpu")[0]):
            import jax.numpy as jnp

            ji, jj = jnp.triu_indices(N, k=1)
            iu = np.asarray(ji).astype(np.int64) % N
            ju = np.asarray(jj).astype(np.int64) % N
        _NC_CACHE["triu"] = (iu, ju)
    return _NC_CACHE["triu"]


def _round_f32r(x):
    """Round-to-nearest-even to 11 explicit mantissa bits (bit-exact match to
    the device's f32 -> f32r conversion, verified empirically)."""
    b = np.ascontiguousarray(x, dtype=np.float32).view(np.uint32)
    drop = 12
    half = np.uint32(1 << (drop - 1))
    mask = np.uint32(~np.uint32((1 << drop) - 1))
    lsb = (b >> drop) & np.uint32(1)
    out = ((b.astype(np.uint64) + (half - np.uint32(1) + lsb)) & mask).astype(np.uint32)
    return out.view(np.float32)


def kernel(feat_embedding, bboxes, w1, b1, w2, b2, wh1, bh1, wh2, bh2,
           wres, bres, wint, bint, wout, bout):
    feat_embedding = np.ascontiguousarray(np.asarray(feat_embedding, dtype=np.float32))
    bboxes = np.ascontiguousarray(np.asarray(bboxes, dtype=np.float32))

    F = feat_embedding.transpose(1, 0, 2).reshape(N, DF)
    B = bboxes.transpose(1, 0, 2).reshape(N, 4)

    wpack = np.zeros((128, WPACK_W), np.float32)
    bint_eff = (np.asarray(bint, np.float32)
                + np.asarray(bres, np.float32) @ np.asarray(wint, np.float32))
    for nm, arr in (("w1", w1), ("w2", w2), ("wh1", wh1), ("wh2", wh2),
                    ("wres", wres), ("wint", wint), ("wout", wout)):
        a = np.asarray(arr, dtype=np.float32)
        wpack[: a.shape[0], _WCOL[nm] : _WCOL[nm] + a.shape[1]] = a
    for nm, arr in (("b1", b1), ("b2", b2), ("bh1", bh1), ("bh2", bh2),
                    ("bres", bres), ("bint", bint_eff), ("bout", bout)):
        a = np.asarray(arr, dtype=np.float32)
        wpack[: a.shape[0], _BCOL[nm]] = a
    ident = np.eye(128, dtype=np.float32)

    in_maps = []
    for c in range(NCORES):
        rows = np.concatenate([
            np.arange(RB * (8 * k + c), RB * (8 * k + c) + RB) for k in range(SLOTS)
        ])
        bb = np.concatenate([B, B[rows]], axis=0)          # [TOT, 4]
        bb = np.ascontiguousarray(bb.reshape(NTT, RB, 4).transpose(1, 0, 2))
        in_maps.append({
            "featxT": _round_f32r(np.concatenate([F, F[rows]], axis=0).T),
            "bboxx": bb,
            "wpack": wpack,
            "ident": ident,
        })

    global _LAST_IN_MAPS
    _LAST_IN_MAPS = in_maps
    res = run_bass_kernel_spmd(_get_nc(), in_maps, list(range(NCORES))).results

    cos = np.empty((N, N), np.float32)
    for c in range(NCORES):
        part = res[c]["cos_part"]
        for k in range(SLOTS):
            r = 8 * k + c
            cos[RB * r : RB * (r + 1), 1024 * k :] = part[RB * k : RB * (k + 1), 1024 * k :]

    # normalize on host exactly as the reference does: cos = gram / max(n_i*n_j, eps)
    # (norms come from the gram diagonal, which every slot computes)
    nrm = np.sqrt(np.ascontiguousarray(np.diagonal(cos)))

    iu, ju = _triu_indices_like_reference()
    # the reference's jnp.triu_indices has fp glitches: some entries are
    # (i+1, -1) or even lower-triangle pairs.  Wrap negatives and read the
    # symmetric entry so every gathered value matches cos[iu, ju] semantics.
    lo = np.minimum(iu, ju)
    hi = np.maximum(iu, ju)
    sims = cos[lo, hi] / np.maximum(nrm[lo] * nrm[hi], np.float32(1e-8))

    loss = np.where(sims == 1.0, np.float32(1.0) - sims, np.float32(0.0)) \
        + np.where(sims == -1.0, np.maximum(sims, np.float32(0.0)), np.float32(0.0))
    loss = np.array(np.sum(loss), dtype=np.float32)
    return loss, sims
